# revision 1
# baseline (speedup 1.0000x reference)
"""Trainium2 Bass kernel for nn_JiuZhouBianMa_26079041421868 (dense_mlp).

out = heads*(1-g) + he*g
  he = concat(heads, pos_codes) @ Wz[h].T   (per-head linear, K=514)
  g  = sigmoid(heads @ Wg.T + bg)

Identity trick: he' = x @ (Wz[h].T - I_pad) = he - heads, so
  out = heads + g * he'  (one fused gate-scale + add on chip).

Sharding: head h -> core h (8 heads, 8 cores, no communication).
Per core: rows = B*S = 16384 over D=512, tiled as 128 row-tiles of 128.

Per row-tile pipeline:
  DMA in -> PE transpose x (4x128x128, fp32) -> ACT cast-copy to MM dtype ->
  PE matmuls (4 k-tiles + padded pos-code tile) into PSUM ->
  gate: DVE mul, ACT accum-copy, ACT sigmoid ->
  blend: ACT scale-by-g from PSUM, DVE add -> DMA out.

MM dtype fp32r (fp32 rounded to 11 mantissa bits, 4x faster on PE).
Set MM_MODE=f32 for exact-fp32 matmuls (~3x slower).
"""
import os
import numpy as np

import concourse.mybir as mybir
import concourse.tile as tile
from concourse import bacc
from concourse.bass import ts
from concourse.bass_utils import run_bass_kernel_spmd
from concourse.masks import make_identity

F32 = mybir.dt.float32
F32R = mybir.dt.float32r

H, B, S, D = 8, 4, 4096, 512
NUM_ZONES = 8
P = 128
ROWS = B * S                    # 16384 rows per core
NT = ROWS // P                  # 128 row-tiles
KT = D // P                     # 4 k-tiles
ST = S // P                     # 32 s-tiles (pos codes repeat per b)

MM_MODE = os.environ.get("MM_MODE", "f32r")


def _build(nc, mm_dt):
    heads_d = nc.dram_tensor("heads", [ROWS, D], F32, kind="ExternalInput").ap()
    wk_d = nc.dram_tensor("wk", [P, KT, D], F32, kind="ExternalInput").ap()
    wpos_d = nc.dram_tensor("wpos", [P, D], F32, kind="ExternalInput").ap()
    pct_d = nc.dram_tensor("pct", [P, S], F32, kind="ExternalInput").ap()
    wgb_d = nc.dram_tensor("wgb", [P, D], F32, kind="ExternalInput").ap()
    bgb_d = nc.dram_tensor("bgb", [P, 1], F32, kind="ExternalInput").ap()
    out_d = nc.dram_tensor("out", [ROWS, D], F32, kind="ExternalOutput").ap()

    with tile.TileContext(nc) as tc:
        with (
            tc.tile_pool(name="const", bufs=1) as cp,
            tc.tile_pool(name="work", bufs=4) as work,
            tc.tile_pool(name="psA", bufs=2, space="PSUM") as psA,
            tc.tile_pool(name="psB", bufs=2, space="PSUM") as psB,
        ):
            ident = cp.tile([P, P], F32)
            make_identity(nc, ident)

            wk_sb = cp.tile([P, KT, D], F32)
            nc.sync.dma_start(wk_sb[:], wk_d)
            wpos_sb = cp.tile([P, D], F32)
            nc.sync.dma_start(wpos_sb[:], wpos_d)
            pct_sb = cp.tile([P, S], F32)
            nc.sync.dma_start(pct_sb[:], pct_d)
            wgb_sb = cp.tile([P, D], F32)
            nc.sync.dma_start(wgb_sb[:], wgb_d)
            bgb_sb = cp.tile([P, 1], F32)
            nc.sync.dma_start(bgb_sb[:], bgb_d)

            if mm_dt == F32R:
                wk_mm = cp.tile([P, KT, D], F32R)
                nc.vector.tensor_copy(wk_mm[:], wk_sb[:])
                wpos_mm = cp.tile([P, D], F32R)
                nc.vector.tensor_copy(wpos_mm[:], wpos_sb[:])
                pct_mm = cp.tile([P, S], F32R)
                nc.vector.tensor_copy(pct_mm[:], pct_sb[:])
            else:
                wk_mm, wpos_mm, pct_mm = wk_sb, wpos_sb, pct_sb

            for t in range(NT):
                st = t % ST  # s-tile index (pos codes repeat across b)

                x_sb = work.tile([P, D], F32, tag="x")
                nc.sync.dma_start(x_sb[:], heads_d[ts(t, P), :])

                xt_ps = psA.tile([P, D], F32, tag="xt")
                for k in range(KT):
                    nc.tensor.transpose(
                        xt_ps[:, ts(k, P)], x_sb[:, ts(k, P)], ident[:]
                    )
                xt_mm = work.tile([P, D], mm_dt, tag="xt_mm")
                nc.scalar.activation(
                    xt_mm[:], xt_ps[:], mybir.ActivationFunctionType.Copy
                )

                he_ps = psB.tile([P, D], F32, tag="he")
                for k in range(KT):
                    nc.tensor.matmul(
                        he_ps[:], xt_mm[:, ts(k, P)], wk_mm[:, k, :],
                        start=(k == 0), stop=False,
                    )
                nc.tensor.matmul(
                    he_ps[:], pct_mm[:, ts(st, P)], wpos_mm[:],
                    start=False, stop=True,
                )

                # gate
                xw = work.tile([P, D], F32, tag="xw")
                nc.vector.tensor_mul(xw[:], x_sb[:], wgb_sb[:])
                g_logit = work.tile([P, 1], F32, tag="gl")
                scr = work.tile([P, D], F32, tag="scr")
                nc.scalar.activation(
                    scr[:], xw[:], mybir.ActivationFunctionType.Copy,
                    accum_out=g_logit[:],
                )
                g_sb = work.tile([P, 1], F32, tag="g")
                nc.scalar.activation(
                    g_sb[:], g_logit[:], mybir.ActivationFunctionType.Sigmoid,
                    bias=bgb_sb[:],
                )

                # blend: out = x + g * he'
                t1 = work.tile([P, D], F32, tag="t1")
                nc.scalar.activation(
                    t1[:], he_ps[:], mybir.ActivationFunctionType.Copy,
                    scale=g_sb[:],
                )
                ob = work.tile([P, D], F32, tag="ob")
                nc.vector.tensor_add(ob[:], t1[:], x_sb[:])
                nc.sync.dma_start(out_d[ts(t, P), :], ob[:])
    return nc


_CACHE = {}


def _get_compiled(mm_mode):
    if mm_mode in _CACHE:
        return _CACHE[mm_mode]
    mm_dt = F32 if mm_mode == "f32" else F32R
    nc = bacc.Bacc("TRN2", target_bir_lowering=False, debug=False,
                   enable_asserts=True, num_devices=8)
    _build(nc, mm_dt)
    nc.compile()
    _CACHE[mm_mode] = nc
    return nc


def _host_prep(heads, Wz, Wg, bg):
    heads = np.ascontiguousarray(heads, dtype=np.float32)
    Wz = np.asarray(Wz, dtype=np.float32)
    Wg = np.asarray(Wg, dtype=np.float32)
    bg = np.asarray(bg, dtype=np.float32)

    # pos codes, computed in fp32 to match the jnp fp32 reference ops
    s = np.arange(S, dtype=np.float32)
    pos = s / np.float32(S - 1)
    zs = np.float32(S / NUM_ZONES)
    zr = (s % zs) / zs
    in_maps = []
    for h in range(H):
        tc_h = np.float32(h) / np.float32(7.0)
        ch0 = pos * np.float32(0.5) + tc_h * np.float32(0.5)
        pct = np.zeros((P, S), dtype=np.float32)
        pct[0] = ch0
        pct[1] = zr

        Wp = Wz[h].T.copy()                       # [514, 512], W'[e, d]
        Wp[np.arange(D), np.arange(D)] -= np.float32(1.0)  # identity trick
        wk = np.ascontiguousarray(
            Wp[:D].reshape(KT, P, D).transpose(1, 0, 2))   # [p, k, d]
        wpos = np.zeros((P, D), dtype=np.float32)
        wpos[:2] = Wp[D:]

        wgb = np.ascontiguousarray(np.broadcast_to(Wg[0], (P, D)))
        bgb = np.full((P, 1), bg[0], dtype=np.float32)

        in_maps.append(dict(
            heads=np.ascontiguousarray(heads[h].reshape(ROWS, D)),
            wk=wk, wpos=wpos, pct=pct, wgb=wgb, bgb=bgb,
        ))
    return in_maps


def run(heads, Wz, Wg, bg, mm_mode=MM_MODE, **spmd_kwargs):
    nc = _get_compiled(mm_mode)
    in_maps = _host_prep(heads, Wz, Wg, bg)
    res = run_bass_kernel_spmd(nc, in_maps, core_ids=list(range(H)),
                               **spmd_kwargs)
    out = np.stack([r["out"].reshape(B, S, D) for r in res.results])
    return out.astype(np.float32), res


def kernel(heads, Wz, Wg, bg):
    out, _ = run(heads, Wz, Wg, bg)
    return out


# revision 2
# speedup vs baseline: 114.0439x; 114.0439x over previous
"""Trainium2 Bass kernel for nn_JiuZhouBianMa_26079041421868 (dense_mlp).

out = heads*(1-g) + he*g
  he = concat(heads, pos_codes) @ Wz[h].T   (per-head linear, K=514)
  g  = sigmoid(heads @ Wg.T + bg)

Identity trick: he' = x @ (Wz[h].T - I_pad) = he - heads, so
  out = heads + g * he'.

Sharding: head h -> core h (8 heads, 8 cores, no communication).
Per core: rows = B*S = 16384 over D=512, processed as 64 pairs of
128-row tiles (pair-batched DMA + wide DVE ops).

Per row-tile pipeline:
  DMA in (2 tiles/DMA) -> PE transpose x (fp32) ->
  cast-copy to fp32r (+ residual split in f32r3 mode) ->
  PE matmuls into PSUM (hi*Whi [+ hi*Wlo + lo*Whi] + padded pos tile) ->
  gate: DVE mul (wide), ACT accum-copy, ACT sigmoid ->
  blend: ACT scale-by-g from PSUM, wide DVE add -> DMA out (2 tiles/DMA).

MM_MODE:
  f32r3 (default): fp32r hi/lo compensated, 3 matmuls per k-tile,
                   ~1e-6 rel err, ~3.3x faster on PE than true fp32.
  f32r:            single-pass fp32r (11-bit mantissa), ~4e-4 rel err,
                   fastest.
  f32:             exact fp32 matmuls (4 cyc/row), slowest.
"""
import os
import numpy as np

import concourse.mybir as mybir
import concourse.tile as tile
from concourse import bacc
from concourse.bass import ts
from concourse.bass_utils import run_bass_kernel_spmd
from concourse.masks import make_identity

F32 = mybir.dt.float32
F32R = mybir.dt.float32r

H, B, S, D = 8, 4, 4096, 512
NUM_ZONES = 8
P = 128
ROWS = B * S                    # 16384 rows per core
NT = ROWS // P                  # 128 row-tiles
NPAIR = NT // 2                 # 64 pair-tiles
KT = D // P                     # 4 k-tiles
ST = S // P                     # 32 s-tiles (pos codes repeat per b)

MM_MODE = os.environ.get("MM_MODE", "f32r3")


def _round_f32r(a):
    """RNE to 11 explicit mantissa bits (matches walrus fp32_to_fp32r)."""
    u = np.ascontiguousarray(a, dtype=np.float32).view(np.uint32)
    lo = u & np.uint32(0xFFF)
    base = u & np.uint32(0xFFFFF000)
    lsb = (u >> np.uint32(12)) & np.uint32(1)
    up = (lo > 0x800) | ((lo == 0x800) & (lsb == 1))
    base = base + np.where(up, np.uint32(0x1000), np.uint32(0))
    return base.view(np.float32).reshape(np.asarray(a).shape)


def _build(nc, mode):
    mm_dt = F32 if mode == "f32" else F32R
    split = mode == "f32r3"

    heads_d = nc.dram_tensor("heads", [ROWS, D], F32, kind="ExternalInput").ap()
    wkh_d = nc.dram_tensor("wkh", [P, KT, D], F32, kind="ExternalInput").ap()
    if split:
        wkl_d = nc.dram_tensor("wkl", [P, KT, D], F32, kind="ExternalInput").ap()
    wpos_d = nc.dram_tensor("wpos", [P, D], F32, kind="ExternalInput").ap()
    pct_d = nc.dram_tensor("pct", [P, S], F32, kind="ExternalInput").ap()
    wgb_d = nc.dram_tensor("wgb", [P, D], F32, kind="ExternalInput").ap()
    bgb_d = nc.dram_tensor("bgb", [P, 1], F32, kind="ExternalInput").ap()
    out_d = nc.dram_tensor("out", [ROWS, D], F32, kind="ExternalOutput").ap()

    heads_pd = heads_d.rearrange("(t a p) d -> t p a d", a=2, p=P)  # [64,128,2,512]
    out_pd = out_d.rearrange("(t a p) d -> t p a d", a=2, p=P)

    with tile.TileContext(nc) as tc:
        with (
            tc.tile_pool(name="const", bufs=1) as cp,
            tc.tile_pool(name="work", bufs=3) as work,
            tc.tile_pool(name="psA", bufs=2, space="PSUM") as psA,
            tc.tile_pool(name="psB", bufs=2, space="PSUM") as psB,
        ):
            ident = cp.tile([P, P], F32)
            make_identity(nc, ident)

            wkh_sb = cp.tile([P, KT, D], F32)
            nc.sync.dma_start(wkh_sb[:], wkh_d)
            wpos_sb = cp.tile([P, D], F32)
            nc.sync.dma_start(wpos_sb[:], wpos_d)
            pct_sb = cp.tile([P, S], F32)
            nc.sync.dma_start(pct_sb[:], pct_d)
            wgb_sb = cp.tile([P, D], F32)
            nc.sync.dma_start(wgb_sb[:], wgb_d)
            bgb_sb = cp.tile([P, 1], F32)
            nc.sync.dma_start(bgb_sb[:], bgb_d)

            if mm_dt == F32R:
                wkh_mm = cp.tile([P, KT, D], F32R)
                nc.vector.tensor_copy(wkh_mm[:], wkh_sb[:])
                wpos_mm = cp.tile([P, D], F32R)
                nc.vector.tensor_copy(wpos_mm[:], wpos_sb[:])
                pct_mm = cp.tile([P, S], F32R)
                nc.vector.tensor_copy(pct_mm[:], pct_sb[:])
                if split:
                    wkl_sb = cp.tile([P, KT, D], F32)
                    nc.sync.dma_start(wkl_sb[:], wkl_d)
                    wkl_mm = cp.tile([P, KT, D], F32R)
                    nc.vector.tensor_copy(wkl_mm[:], wkl_sb[:])
            else:
                wkh_mm, wpos_mm, pct_mm = wkh_sb, wpos_sb, pct_sb

            for t in range(NPAIR):
                x2 = work.tile([P, 2, D], F32, tag="x2")
                nc.sync.dma_start(x2[:], heads_pd[t])

                # transposes: both tiles into one 2-bank psum buffer
                xt_ps = psA.tile([P, 2, D], F32, tag="xt")
                for j in range(2):
                    for k in range(KT):
                        nc.tensor.transpose(
                            xt_ps[:, j, ts(k, P)], x2[:, j, ts(k, P)], ident[:]
                        )
                # hi part (rounding cast) on ACT, wide
                xt_hi = work.tile([P, 2, D], mm_dt, tag="xt_hi")
                nc.scalar.activation(
                    xt_hi[:], xt_ps[:], mybir.ActivationFunctionType.Copy
                )
                if split:
                    # residual: lo = round_f32r(x - hi), wide DVE
                    xt_lo = work.tile([P, 2, D], F32R, tag="xt_lo")
                    nc.vector.tensor_tensor(
                        xt_lo[:], xt_ps[:], xt_hi[:], mybir.AluOpType.subtract
                    )

                he_ps = psB.tile([P, 2, D], F32, tag="he")
                for j in range(2):
                    st = (2 * t + j) % ST
                    for k in range(KT):
                        nc.tensor.matmul(
                            he_ps[:, j, :], xt_hi[:, j, ts(k, P)], wkh_mm[:, k, :],
                            start=(k == 0), stop=False,
                        )
                    if split:
                        for k in range(KT):
                            nc.tensor.matmul(
                                he_ps[:, j, :], xt_hi[:, j, ts(k, P)],
                                wkl_mm[:, k, :], start=False, stop=False,
                            )
                            nc.tensor.matmul(
                                he_ps[:, j, :], xt_lo[:, j, ts(k, P)],
                                wkh_mm[:, k, :], start=False, stop=False,
                            )
                    nc.tensor.matmul(
                        he_ps[:, j, :], pct_mm[:, ts(st, P)], wpos_mm[:],
                        start=False, stop=True,
                    )

                # gate: logits via wide DVE mul + per-tile ACT accum-copy
                xw = work.tile([P, 2, D], F32, tag="xw")
                nc.vector.tensor_tensor(
                    xw[:], x2[:], wgb_sb[:, None, :].to_broadcast((P, 2, D)),
                    mybir.AluOpType.mult,
                )
                g_sb = work.tile([P, 2], F32, tag="g")
                scr = work.tile([P, 2, D], F32, tag="scr")
                for j in range(2):
                    gl = work.tile([P, 1], F32, tag=f"gl{j}")
                    nc.scalar.activation(
                        scr[:, j, :], xw[:, j, :],
                        mybir.ActivationFunctionType.Copy, accum_out=gl[:],
                    )
                    nc.scalar.activation(
                        g_sb[:, j : j + 1], gl[:],
                        mybir.ActivationFunctionType.Sigmoid, bias=bgb_sb[:],
                    )

                # blend: t1 = g * he' (per-tile ACT), out = x + t1 (wide DVE)
                t1 = work.tile([P, 2, D], F32, tag="t1")
                for j in range(2):
                    nc.scalar.activation(
                        t1[:, j, :], he_ps[:, j, :],
                        mybir.ActivationFunctionType.Copy,
                        scale=g_sb[:, j : j + 1],
                    )
                ob = work.tile([P, 2, D], F32, tag="ob")
                nc.vector.tensor_add(ob[:], t1[:], x2[:])
                nc.sync.dma_start(out_pd[t], ob[:])
    return nc


_CACHE = {}


def _get_compiled(mm_mode):
    if mm_mode in _CACHE:
        return _CACHE[mm_mode]
    nc = bacc.Bacc("TRN2", target_bir_lowering=False, debug=False,
                   enable_asserts=True, num_devices=8)
    _build(nc, mm_mode)
    nc.compile()
    _CACHE[mm_mode] = nc
    return nc


def _host_prep(heads, Wz, Wg, bg, split):
    heads = np.ascontiguousarray(heads, dtype=np.float32)
    Wz = np.asarray(Wz, dtype=np.float32)
    Wg = np.asarray(Wg, dtype=np.float32)
    bg = np.asarray(bg, dtype=np.float32)

    # pos codes, computed in fp32 to match the jnp fp32 reference ops
    s = np.arange(S, dtype=np.float32)
    pos = s / np.float32(S - 1)
    zs = np.float32(S / NUM_ZONES)
    zr = (s % zs) / zs
    in_maps = []
    for h in range(H):
        tc_h = np.float32(h) / np.float32(7.0)
        ch0 = pos * np.float32(0.5) + tc_h * np.float32(0.5)
        pct = np.zeros((P, S), dtype=np.float32)
        pct[0] = ch0
        pct[1] = zr

        Wp = Wz[h].T.copy()                       # [514, 512], W'[e, d]
        Wp[np.arange(D), np.arange(D)] -= np.float32(1.0)  # identity trick
        wmain = Wp[:D]
        if split:
            w_hi = _round_f32r(wmain)
            w_lo = _round_f32r(wmain - w_hi)
            wkh = np.ascontiguousarray(
                w_hi.reshape(KT, P, D).transpose(1, 0, 2))
            wkl = np.ascontiguousarray(
                w_lo.reshape(KT, P, D).transpose(1, 0, 2))
        else:
            wkh = np.ascontiguousarray(
                wmain.reshape(KT, P, D).transpose(1, 0, 2))
            wkl = None
        wpos = np.zeros((P, D), dtype=np.float32)
        wpos[:2] = Wp[D:]

        wgb = np.ascontiguousarray(np.broadcast_to(Wg[0], (P, D)))
        bgb = np.full((P, 1), bg[0], dtype=np.float32)

        m = dict(
            heads=np.ascontiguousarray(heads[h].reshape(ROWS, D)),
            wkh=wkh, wpos=wpos, pct=pct, wgb=wgb, bgb=bgb,
        )
        if split:
            m["wkl"] = wkl
        in_maps.append(m)
    return in_maps


def run(heads, Wz, Wg, bg, mm_mode=MM_MODE, **spmd_kwargs):
    nc = _get_compiled(mm_mode)
    in_maps = _host_prep(heads, Wz, Wg, bg, split=(mm_mode == "f32r3"))
    res = run_bass_kernel_spmd(nc, in_maps, core_ids=list(range(H)),
                               **spmd_kwargs)
    out = np.stack([r["out"].reshape(B, S, D) for r in res.results])
    return out.astype(np.float32), res


def kernel(heads, Wz, Wg, bg):
    out, _ = run(heads, Wz, Wg, bg)
    return out
